# revision 1
# baseline (speedup 1.0000x reference)
"""Trainium2 Bass kernel for nn_ExplodedLogit (topk_masking).

Reference computation (x (512,256) f32, W (1,256) f32, b (1,) f32):
    scores = x @ W.T + b                                  (512, 1)
    idx    = argmax(scores)
    mask   = ones(512) with log(1e-46) at idx
    block  = scores * mask[None, :]                       (512, 512)
    out    = concat([scores, tile(block, (1, 512))], 1)   (512, 262145)

Sharding: the 512 identical block repetitions are split across 8
NeuronCores, 64 reps each -> per-core "rep" output (512, 32768) = 64 MB
(memory-bound: this is an HBM-write problem). Every core runs the
identical program: scores/argmax/mask are recomputed redundantly (tiny),
and the per-core slice is materialized with fan-out DMAs that read a
small SBUF block through a step-0 (broadcast) access-pattern dim.

Row layout: r = 128t + p (p = partition 0..127, t = 0..3). The
cross-partition broadcast of the 512 scores (needed to build the mask
along the free dim) runs entirely on-chip: PE transposes scores
[128,4] -> [4,128], then four selector matmuls broadcast each
128-score chunk to all partitions in PSUM — no DRAM round-trip.
"""

import math

import numpy as np

import concourse.bacc as bacc
import concourse.bass_utils as _bass_utils
import concourse.mybir as mybir
import concourse.tile as tile
from concourse.bass_utils import run_bass_kernel_spmd

# If profiling is enabled via env (BASS_TRACE), a failed artifact upload
# must not take down the run — fall back to the local tmpdir.
_orig_upload = _bass_utils.upload_artifacts


def _safe_upload(tmpdir):
    try:
        return _orig_upload(tmpdir)
    except Exception:
        return tmpdir


_bass_utils.upload_artifacts = _safe_upload

F32 = mybir.dt.float32
MASK_VAL = float(np.float32(math.log(1e-46)))  # ~ -105.9189

T = 512        # tracks (rows)
F = 256        # features
P = 128        # SBUF partitions
TPP = T // P   # 4 rows per partition (r = 128t + p)
NREP = 512     # total block repetitions in the full output
NCORES = 8
RPC = NREP // NCORES   # 64 reps per core
R = 8                  # reps materialized in SBUF
G = RPC // R           # step-0 groups per fan-out DMA


def _build():
    nc = bacc.Bacc("TRN2", target_bir_lowering=False, debug=False)
    x = nc.dram_tensor("x", [T, F], F32, kind="ExternalInput")
    W = nc.dram_tensor("W", [1, F], F32, kind="ExternalInput")
    b = nc.dram_tensor("b", [1, 1], F32, kind="ExternalInput")
    rep_out = nc.dram_tensor("rep", [T, RPC * T], F32, kind="ExternalOutput")
    scores_out = nc.dram_tensor("scores", [T, 1], F32, kind="ExternalOutput")

    with tile.TileContext(nc) as tc:
        with (
            tc.tile_pool(name="sbuf", bufs=1) as sbuf_pool,
            tc.tile_pool(name="psum", bufs=1, space="PSUM") as psum_pool,
        ):
            _emit(nc, x[:], W[:], b[:], rep_out[:], scores_out[:],
                  sbuf_pool, psum_pool)
    nc.compile()
    return nc


def _emit(nc, x, W, b, rep_out, scores_out, sbuf_pool, psum_pool):
    x_sb = sbuf_pool.tile([P, TPP * F], F32)     # x[128t+p, f] at [p, t*F+f]
    w_sb = sbuf_pool.tile([P, F], F32)
    b_sb = sbuf_pool.tile([P, 1], F32)
    tmp_sb = sbuf_pool.tile([P, TPP * F], F32)
    sc_sb = sbuf_pool.tile([P, TPP], F32)        # scores: s[128t+p] at [p,t]
    ones_sb = sbuf_pool.tile([P, P], F32)
    id_sb = sbuf_pool.tile([P, P], F32)          # 128x128 identity
    onesk_sb = sbuf_pool.tile([TPP, TPP * P], F32)
    sel_sb = sbuf_pool.tile([TPP, TPP * P], F32)  # selector one-hot rows
    s4_sb = sbuf_pool.tile([TPP, P], F32)        # scores, free-dim chunks
    m8_sb = sbuf_pool.tile([P, 8], F32)
    mask_sb = sbuf_pool.tile([P, T], F32)
    rep_sb = sbuf_pool.tile([P, TPP * R * T], F32)

    sT_ps = psum_pool.tile([TPP, P], F32)
    sbc_ps = psum_pool.tile([P, T], F32)

    # ---- constants (overlap with the x load) ----
    nc.vector.memset(ones_sb[:], 1.0)
    # identity: keep ones where (col - row) == 0
    nc.gpsimd.affine_select(
        id_sb[:], ones_sb[:], [[1, P]], mybir.AluOpType.is_equal, 0.0,
        base=0, channel_multiplier=-1,
    )
    # selector: sel[k, t*P + m] = 1 iff k == t  (iota val = t - k)
    nc.vector.memset(onesk_sb[:], 1.0)
    nc.gpsimd.affine_select(
        sel_sb[:].rearrange("k (t m) -> k t m", t=TPP),
        onesk_sb[:].rearrange("k (t m) -> k t m", t=TPP),
        [[1, TPP], [0, P]], mybir.AluOpType.is_equal, 0.0,
        base=0, channel_multiplier=-1,
    )

    # ---- loads ----
    # inputs split across both HWDGE rings (SP + ACT) so descriptor
    # generation and completion receipts run in parallel
    nc.sync.dma_start(b_sb[:], b.broadcast_to((P, 1)))
    nc.scalar.dma_start(w_sb[:], W.broadcast_to((P, F)))
    # x in 4 row-chunks so scores compute overlaps the tail of the load
    for t in range(TPP):
        eng = nc.sync if t % 2 == 0 else nc.scalar
        eng.dma_start(
            x_sb[:, t * F:(t + 1) * F], x[128 * t:128 * (t + 1), :]
        )

    # ---- scores: s[128t+p] = b + sum_f x[128t+p,f] * W[f] ----
    # (tensor_tensor_reduce would fuse mul+reduce, but it hard-crashes
    #  the device here — NRT_EXEC_UNIT_UNRECOVERABLE — so two ops.)
    for t in range(TPP):
        nc.vector.tensor_mul(
            tmp_sb[:, t * F:(t + 1) * F],
            x_sb[:, t * F:(t + 1) * F],
            w_sb[:],
        )
        nc.vector.reduce_sum(
            sc_sb[:, t:t + 1], tmp_sb[:, t * F:(t + 1) * F],
            axis=mybir.AxisListType.X,
        )

    # ---- broadcast scores to all partitions, on-chip (PE) ----
    # transpose PRE-bias scores (argmax is shift-invariant); the bias is
    # folded into the PSUM->SBUF copy, and added to sc_sb in parallel.
    # transpose: sT[t, p] = sc[p, t] = s[128t+p] - b
    nc.tensor.matmul(sT_ps[:], lhsT=sc_sb[:], rhs=id_sb[:])
    nc.vector.tensor_scalar_add(sc_sb[:], sc_sb[:], b_sb[:, 0:1])
    nc.vector.tensor_scalar_add(s4_sb[:], sT_ps[:], b_sb[0:TPP, 0:1])
    # external scores output (off the critical path)
    nc.sync.dma_start(
        scores_out.rearrange("(t p) one -> t (p one)", t=TPP), s4_sb[:]
    )
    # sbc[:, t*P:(t+1)*P] = sel_t.T @ s4 -> every partition gets chunk t
    for t in range(TPP):
        nc.tensor.matmul(
            sbc_ps[:, t * P:(t + 1) * P],
            lhsT=sel_sb[:, t * P:(t + 1) * P],
            rhs=s4_sb[:],
        )

    # ---- mask, read straight from PSUM (no copy) ----
    nc.vector.max(m8_sb[:], sbc_ps[:])
    # ind = (s == max); mask = ind * (MASK_VAL-1) + 1  (exact in f32 here)
    nc.vector.tensor_scalar(
        mask_sb[:], sbc_ps[:], m8_sb[:, 0:1], None, mybir.AluOpType.is_equal
    )
    nc.vector.tensor_scalar(
        mask_sb[:], mask_sb[:], MASK_VAL - 1.0, 1.0,
        mybir.AluOpType.mult, mybir.AluOpType.add,
    )

    # ---- fill rep_sb: R copies of each row's block slice ----
    # rep_sb[p, (t*R+r)*T + c] = sc[p,t] * mask[c]
    # t=0 gates the first fan-out DMA: fill its halves on DVE and ACT in
    # parallel and write them with separate DMAs so streaming starts after
    # half a fill. t=1..3 overlap with streaming anyway.
    h = R // 2
    nc.vector.tensor_scalar(
        rep_sb[:, 0:h * T].rearrange("p (r c) -> p r c", c=T),
        mask_sb[:].unsqueeze(1).broadcast_to((P, h, T)),
        sc_sb[:, 0:1], None, mybir.AluOpType.mult,
    )
    nc.scalar.activation(
        rep_sb[:, h * T:R * T].rearrange("p (r c) -> p r c", c=T),
        mask_sb[:].unsqueeze(1).broadcast_to((P, h, T)),
        mybir.ActivationFunctionType.Copy,
        scale=sc_sb[:, 0:1],
    )
    for t in range(1, TPP):
        nc.vector.tensor_scalar(
            rep_sb[:, t * R * T:(t + 1) * R * T].rearrange(
                "p (r c) -> p r c", c=T
            ),
            mask_sb[:].unsqueeze(1).broadcast_to((P, R, T)),
            sc_sb[:, t:t + 1], None, mybir.AluOpType.mult,
        )

    # ---- fan-out DMAs: write each t-slot G times via a step-0 src dim ----
    out_v = rep_out.rearrange("(t p) (g q) -> t p g q", p=P, q=R * T)
    # t=0 in rep-halves so the first write only waits for half a fill
    for half in range(2):
        src = (
            rep_sb[:, half * h * T:(half + 1) * h * T]
            .unsqueeze(1)
            .broadcast_to((P, G, h * T))
        )
        dst = rep_out.rearrange(
            "(t p) (g u) -> t p g u", p=P, u=R * T
        )[0][:, :, half * h * T:(half + 1) * h * T]
        nc.sync.dma_start(dst, src)
    for t in range(1, TPP):
        src = (
            rep_sb[:, t * R * T:(t + 1) * R * T]
            .unsqueeze(1)
            .broadcast_to((P, G, R * T))
        )
        nc.sync.dma_start(out_v[t], src)


_NC_CACHE = None


def _get_nc():
    global _NC_CACHE
    if _NC_CACHE is None:
        _NC_CACHE = _build()
    return _NC_CACHE


def _run(x, W, b, **run_kwargs):
    nc = _get_nc()
    in_map = {
        "x": np.ascontiguousarray(np.asarray(x, dtype=np.float32)),
        "W": np.ascontiguousarray(np.asarray(W, dtype=np.float32)).reshape(1, F),
        "b": np.ascontiguousarray(np.asarray(b, dtype=np.float32)).reshape(1, 1),
    }
    # The device pool occasionally throws a transient
    # NRT_EXEC_UNIT_UNRECOVERABLE on dispatch; a retry lands cleanly.
    last_err = None
    for attempt in range(3):
        try:
            return run_bass_kernel_spmd(
                nc,
                [dict(in_map) for _ in range(NCORES)],
                core_ids=list(range(NCORES)),
                **run_kwargs,
            )
        except Exception as e:  # noqa: BLE001
            last_err = e
            import time
            time.sleep(2.0 * (attempt + 1))
            try:
                import jax
                jax.clear_caches()
                jax.clear_backends()
            except Exception:
                pass
    raise last_err


def kernel(x, W, b):
    res = _run(x, W, b)
    outs = res.results
    full = np.empty((T, 1 + NREP * T), dtype=np.float32)
    full[:, 0:1] = outs[0]["scores"]
    for c in range(NCORES):
        full[:, 1 + c * RPC * T: 1 + (c + 1) * RPC * T] = outs[c]["rep"]
    return full



# revision 5
# speedup vs baseline: 1.0972x; 1.0972x over previous
"""Trainium2 Bass kernel for nn_ExplodedLogit (topk_masking).

Reference computation (x (512,256) f32, W (1,256) f32, b (1,) f32):
    scores = x @ W.T + b                                  (512, 1)
    idx    = argmax(scores)
    mask   = ones(512) with log(1e-46) at idx
    block  = scores * mask[None, :]                       (512, 512)
    out    = concat([scores, tile(block, (1, 512))], 1)   (512, 262145)

Sharding: the 512 identical block repetitions are split across 8
NeuronCores, 64 reps each -> per-core "rep" output (512, 32768) = 64 MB
(memory-bound: an HBM-write problem; the stream runs at the ~352 GB/s
per-core HBM wall). Every core runs the identical program: scores/
argmax/mask are recomputed redundantly (tiny) and the per-core slice is
materialized with fan-out DMAs that read a small SBUF block through a
step-0 (broadcast) access-pattern dim.

The only optimizable term is how early the fan-out stream starts, so the
prologue is organized around the critical chain  x -> scores -> mask ->
first fill -> first descriptor:

* Row layout r = 4p + t (p = partition, t = 0..3): each partition's 4
  rows are CONTIGUOUS in DRAM, so x loads with 2 KB descriptors in two
  chunks (one per HWDGE ring) instead of 1 KB ones.
* scores: DVE does t=0,1 (mul+reduce), GpSimd does t=2,3 in parallel.
* Cross-partition broadcast of the 512 scores runs as ONE PE matmul:
  GpSimd builds diag[q, (m,t)] = sc[q,t] * (m==q), then
  ones[128,128].T @ diag = sbc[p, c] = s[c] on every partition
  (c = 4m+t matches the output-column order because m is the outer
  free dim). This replaces transpose + 4 selector matmuls.
* mask is fused: indm = (sbc == max) * (MASK_VAL-1) in one dual-op
  tensor_scalar; each fill computes (indm + 1) * sc in one dual-op.
* Only R=2 reps per t are materialized (4 KB/partition); each t's
  fan-out DMA (G=32 step-0 copies, 4 KB descriptors) is gated on its
  own small fill. t0/t2 go on the sync HWDGE ring, t1/t3 on scalar.
  At >=4 KB descriptors the SDMA engines stay at the HBM wall.
* scores output is one PSUM row copied to SBUF [1,512] and DMA'd with
  a single descriptor, queued last so its completion receipt lands
  mid-stream.
"""

import math

import numpy as np

import concourse.bacc as bacc
import concourse.bass_utils as _bass_utils
import concourse.mybir as mybir
import concourse.tile as tile
from concourse.bass_utils import run_bass_kernel_spmd

# If profiling is enabled via env (BASS_TRACE), a failed artifact upload
# must not take down the run — fall back to the local tmpdir.
_orig_upload = _bass_utils.upload_artifacts


def _safe_upload(tmpdir):
    try:
        return _orig_upload(tmpdir)
    except Exception:
        return tmpdir


_bass_utils.upload_artifacts = _safe_upload

F32 = mybir.dt.float32
MASK_VAL = float(np.float32(math.log(1e-46)))  # ~ -105.9189

T = 512        # tracks (rows)
F = 256        # features
P = 128        # SBUF partitions
TPP = T // P   # 4 rows per partition (r = 4p + t)
NREP = 512     # total block repetitions in the full output
NCORES = 8
RPC = NREP // NCORES   # 64 reps per core
R2 = 2                 # reps materialized in SBUF per t
G2 = RPC // R2         # step-0 copies per fan-out DMA


def _build():
    nc = bacc.Bacc("TRN2", target_bir_lowering=False, debug=False)
    x = nc.dram_tensor("x", [T, F], F32, kind="ExternalInput")
    W = nc.dram_tensor("W", [1, F], F32, kind="ExternalInput")
    b = nc.dram_tensor("b", [1, 1], F32, kind="ExternalInput")
    rep_out = nc.dram_tensor("rep", [T, RPC * T], F32, kind="ExternalOutput")
    scores_out = nc.dram_tensor("scores", [T, 1], F32, kind="ExternalOutput")

    with tile.TileContext(nc) as tc:
        with (
            tc.tile_pool(name="sbuf", bufs=1) as sbuf_pool,
            tc.tile_pool(name="psum", bufs=1, space="PSUM") as psum_pool,
        ):
            _emit(nc, x[:], W[:], b[:], rep_out[:], scores_out[:],
                  sbuf_pool, psum_pool)
    nc.compile()
    return nc


def _emit(nc, x, W, b, rep_out, scores_out, sbuf_pool, psum_pool):
    x_sb = sbuf_pool.tile([P, TPP * F], F32)     # x[4p+t, f] at [p, t*F+f]
    w_sb = sbuf_pool.tile([P, F], F32)
    b_sb = sbuf_pool.tile([P, 1], F32)
    tmp_sb = sbuf_pool.tile([P, TPP * F], F32)
    sc_sb = sbuf_pool.tile([P, TPP], F32)        # scores: s[4p+t] at [p,t]
    ones_sb = sbuf_pool.tile([P, P], F32)
    diag_sb = sbuf_pool.tile([P, P * TPP], F32)  # sc[q,t] at [q, 4q+t]
    m8_sb = sbuf_pool.tile([P, 8], F32)
    indm_sb = sbuf_pool.tile([P, T], F32)        # (s==max)*(MASK_VAL-1)
    rep_sb = sbuf_pool.tile([P, TPP * R2 * T], F32)
    srow_sb = sbuf_pool.tile([1, T], F32)

    sbc_ps = psum_pool.tile([P, T], F32)

    # ---- constants (overlap with the loads) ----
    nc.vector.memset(ones_sb[:], 1.0)

    # ---- loads ----
    # Two x chunks split across both HWDGE rings; per-partition rows
    # 4p..4p+3 are contiguous in DRAM -> 2 KB descriptors per chunk.
    x_v = x.rearrange("(p t) f -> p t f", t=TPP)
    h = TPP // 2
    nc.scalar.dma_start(w_sb[:], W.broadcast_to((P, F)))
    nc.scalar.dma_start(
        x_sb[:, 0:h * F].rearrange("p (t f) -> p t f", f=F), x_v[:, 0:h]
    )
    nc.sync.dma_start(
        x_sb[:, h * F:].rearrange("p (t f) -> p t f", f=F), x_v[:, h:]
    )
    nc.sync.dma_start(b_sb[:], b.broadcast_to((P, 1)))

    # ---- scores: s[4p+t] = b + sum_f x[4p+t,f] * W[f] ----
    # One scalar_tensor_tensor per t: out = (x*1)*W, accum_out = row
    # sum. Vector-only: Pool fails the codegen engine check for stt,
    # and tensor_tensor_reduce hard-crashes the device
    # (NRT_EXEC_UNIT_UNRECOVERABLE) — stt is a native
    # InstTensorScalarPtr and is fine on DVE.
    for t in range(TPP):
        eng = nc.vector
        eng.scalar_tensor_tensor(
            tmp_sb[:, t * F:(t + 1) * F],
            x_sb[:, t * F:(t + 1) * F],
            1.0,
            w_sb[:],
            mybir.AluOpType.mult,
            mybir.AluOpType.mult,
            accum_out=sc_sb[:, t:t + 1],
        )
    nc.vector.tensor_scalar_add(sc_sb[:], sc_sb[:], b_sb[:, 0:1])

    # ---- broadcast scores to all partitions: one PE matmul ----
    # diag[q, (m, t)] = sc[q, t] iff m == q  (iota val = m - q)
    nc.gpsimd.affine_select(
        diag_sb[:].rearrange("q (m t) -> q m t", t=TPP),
        sc_sb[:].unsqueeze(1).broadcast_to((P, P, TPP)),
        [[1, P], [0, TPP]], mybir.AluOpType.is_equal, 0.0,
        base=0, channel_multiplier=-1,
    )
    # sbc[p, (m, t)] = sum_q diag[q, (m, t)] = s[4m + t] = s[c]
    nc.tensor.matmul(sbc_ps[:], lhsT=ones_sb[:], rhs=diag_sb[:])

    # ---- mask term, read straight from PSUM ----
    nc.vector.max(m8_sb[:], sbc_ps[:])
    # indm = (s == max) * (MASK_VAL-1);  fill does (indm + 1) * sc
    nc.vector.tensor_scalar(
        indm_sb[:], sbc_ps[:], m8_sb[:, 0:1], MASK_VAL - 1.0,
        mybir.AluOpType.is_equal, mybir.AluOpType.mult,
    )

    # ---- fills + fan-out DMAs, one per t ----
    # rep_sb[p, (t r c)] = sc[p,t] * mask[c];  DVE fills t0,t1,t2 (it is
    # ~2x faster than GpSimd), GpSimd fills t3 in parallel.
    out_v = rep_out.rearrange("(p t) (g u) -> t p g u", t=TPP, u=R2 * T)
    fill_eng = {0: nc.vector, 1: nc.vector, 2: nc.vector, 3: nc.gpsimd}
    dma_eng = {0: nc.sync, 1: nc.scalar, 2: nc.sync, 3: nc.scalar}
    for t in (0, 3, 1, 2):
        fill_eng[t].tensor_scalar(
            rep_sb[:, t * R2 * T:(t + 1) * R2 * T].rearrange(
                "p (r c) -> p r c", c=T
            ),
            indm_sb[:].unsqueeze(1).broadcast_to((P, R2, T)),
            1.0, sc_sb[:, t:t + 1],
            mybir.AluOpType.add, mybir.AluOpType.mult,
        )
    for t in (0, 3, 1, 2):
        src = (
            rep_sb[:, t * R2 * T:(t + 1) * R2 * T]
            .unsqueeze(1)
            .broadcast_to((P, G2, R2 * T))
        )
        dma_eng[t].dma_start(out_v[t], src)

    # ---- scores output: one PSUM row -> SBUF [1,512] -> 1 descriptor ----
    # Queued last on the scalar ring: its descriptors wait behind the
    # rep stream, so the completion receipt lands mid-stream, not at
    # the end of the kernel.
    nc.vector.tensor_scalar_add(srow_sb[:], sbc_ps[0:1, :], 0.0)
    nc.scalar.dma_start(
        scores_out.rearrange("t one -> one t"), srow_sb[:]
    )


_NC_CACHE = None


def _get_nc():
    global _NC_CACHE
    if _NC_CACHE is None:
        _NC_CACHE = _build()
    return _NC_CACHE


def _run(x, W, b, **run_kwargs):
    nc = _get_nc()
    in_map = {
        "x": np.ascontiguousarray(np.asarray(x, dtype=np.float32)),
        "W": np.ascontiguousarray(np.asarray(W, dtype=np.float32)).reshape(1, F),
        "b": np.ascontiguousarray(np.asarray(b, dtype=np.float32)).reshape(1, 1),
    }
    # The device pool occasionally throws a transient
    # NRT_EXEC_UNIT_UNRECOVERABLE on dispatch; a retry lands cleanly.
    last_err = None
    for attempt in range(3):
        try:
            return run_bass_kernel_spmd(
                nc,
                [dict(in_map) for _ in range(NCORES)],
                core_ids=list(range(NCORES)),
                **run_kwargs,
            )
        except Exception as e:  # noqa: BLE001
            last_err = e
            import time
            time.sleep(2.0 * (attempt + 1))
            try:
                import jax
                jax.clear_caches()
                jax.clear_backends()
            except Exception:
                pass
    raise last_err


def kernel(x, W, b):
    res = _run(x, W, b)
    outs = res.results
    full = np.empty((T, 1 + NREP * T), dtype=np.float32)
    full[:, 0:1] = outs[0]["scores"]
    for c in range(NCORES):
        full[:, 1 + c * RPC * T: 1 + (c + 1) * RPC * T] = outs[c]["rep"]
    return full
